# revision 25
# baseline (speedup 1.0000x reference)
"""GRNN (graph recurrent NN) Trainium2 Bass kernel.

Reference computation (per batch item b, T=64 steps, Z0 = 0):
    Z_{t+1} = tanh(S @ Z_t @ W + x_t @ A)      (N,H)
    u_t     = Z_{t+1} @ B                      (N,Q)
    x_{t+1} = x_t + u_t                        (N,P)
Outputs: x_traj (B,T+1,N,P), u_traj (B,T,N,Q).

Sharding: data-parallel over batch across 8 cores (2 items/core). The two
items are folded into a single 128-wide axis r = item*H + h, with
block-diagonal weights Wd/Ad/Bd, so every matmul uses full 128 partitions.

Layout trick: the per-core state is V_t = Z_t @ Wd kept NODE-major.  Then
    (S @ Z_t @ Wd)^T = (S @ V_t)^T      -> matmul(lhsT=V[jchunk], rhs=ST[jchunk])
which lands h-major, where the Wd/Ad/Bd contractions want it, and
    V_{t+1} = Z_{t+1} @ Wd              -> matmul(lhsT=Znew^T[nchunk], rhs=Wd)
lands node-major again.  No transposes anywhere in the loop.
"""

import os
import sys

import numpy as np

for _p in (
    "/root/.axon_site",
    "/root/.axon_site/_ro/trn_rl_repo",
    "/root/.axon_site/_ro/pypackages",
    "/opt/trn_rl_repo",
):
    if os.path.isdir(_p) and _p not in sys.path:
        sys.path.append(_p)

import concourse.bass as bass
import concourse.tile as tile
from concourse import bacc, mybir
from concourse.bass_utils import run_bass_kernel_spmd

BATCH, N, T, P, Q, H = 16, 1024, 64, 8, 8, 64
NCORES = 8
IPC = BATCH // NCORES  # items per core = 2
R = IPC * H            # folded item*H axis = 128
S16 = IPC * P          # folded item*P axis = 16
NCHUNKS = N // 128     # 8 node chunks
F = 512                # matmul moving(free)-dim chunk
NF = N // F            # 2 free chunks over nodes

f32 = mybir.dt.float32

f16 = mybir.dt.float16

# Matmul operand mode:
#   None    — exact fp32 matmuls (4 cyc/row on PE): ~1.27 ms measured.
#   "fp16"  — fp16 operands (1 cyc/row, 11-bit mantissa).  All matmul
#             operands (S^T, V, Znew, Wd/Ad/Bd, x-shadow) stored fp16;
#             psum accumulation and the x trajectory state stay fp32.
#   "m1split" — S-matmul in 3-pass split-fp16 (hi/lo, lo prescaled 2^11,
#             ~22-bit effective operands); M3/M4/M2' stay exact fp32.
#             The recurrence amplifies per-step rounding ~64x (integrator
#             feedback), so plain fp16/f32r operands land at ~2e-2 final
#             error; the split keeps it ~1e-4 while cutting the dominant
#             matmul from 4 to 3 cyc/row total.
#   float32r (bitcast mode) exists in the code but yields wrong results on
#   HW via this Bacc path (standalone-ldweights f32r bug) — do not use.
FAST_MODE = "m1split"
LOSCALE = 2048.0


def _cast(ap, dt):
    return ap if dt is None else ap.bitcast(dt)


def build_nc(mode=FAST_MODE):
    # Bacc (not raw Bass): its compile() splits sync waits to the 1-per-
    # instruction hardware limit and moves matmul waits onto ldweights.
    nc = bacc.Bacc(trn_type="TRN2", target_bir_lowering=False, debug=False)

    wdt = f16 if mode == "fp16" else f32
    if mode == "m1split":
        ST = nc.dram_tensor("ST", [2, N, N], f16, kind="ExternalInput").ap()
        Wd = nc.dram_tensor("Wd", [2, R, R], f16, kind="ExternalInput").ap()
        Bd = nc.dram_tensor("Bd", [2, R, S16], f16, kind="ExternalInput").ap()
    else:
        ST = nc.dram_tensor("ST", [N, N], wdt, kind="ExternalInput").ap()
        Wd = nc.dram_tensor("Wd", [R, R], wdt, kind="ExternalInput").ap()
        Bd = nc.dram_tensor("Bd", [R, S16], wdt, kind="ExternalInput").ap()
    Ad = nc.dram_tensor("Ad", [R, R], f32 if mode == "m1split" else wdt,
                        kind="ExternalInput").ap()
    X0T = nc.dram_tensor("X0T", [128, N], f32, kind="ExternalInput").ap()
    xout = nc.dram_tensor("xout", [T + 1, S16, N], f32, kind="ExternalOutput").ap()
    uout = nc.dram_tensor("uout", [T, S16, N], f32, kind="ExternalOutput").ap()

    with tile.TileContext(nc) as tc:
        _grnn(tc, ST, Wd, Ad, Bd, X0T, xout, uout, mode)
    nc.compile()
    return nc


def _grnn(tc, ST, Wd, Ad, Bd, X0T, xout, uout, mode):
    nc = tc.nc
    fp16 = mode == "fp16"
    m1s = mode == "m1split"
    wdt = f16 if fp16 else f32           # matmul operand storage dtype
    fast_dt = None if (fp16 or m1s or mode is None) else mode  # f32r bitcast
    Tanh = mybir.ActivationFunctionType.Tanh

    with (
        tc.tile_pool(name="singles", bufs=1) as singles,
        tc.tile_pool(name="znew", bufs=2) as zpool,
        tc.tile_pool(name="usb", bufs=2) as upool,
        tc.tile_pool(name="arg", bufs=2) as apool,
        tc.tile_pool(name="vtmp", bufs=2) as vpool,
        tc.tile_pool(name="pP", bufs=2, space="PSUM") as pP,
        tc.tile_pool(name="pU", bufs=2, space="PSUM") as pU,
        tc.tile_pool(name="pV", bufs=2, space="PSUM") as pV,
        tc.tile_pool(name="pLO", bufs=2, space="PSUM") as pLO,
    ):
        # --- resident tensors -------------------------------------------------
        # DMA loads land in staging tiles; one-time DVE copies produce the
        # resident operand tiles.  PE matmuls must carry at most ONE sync wait
        # (LW-struct codegen limit), and Tile's wait elision is not transitive
        # across procs — so matmul operands must be produced by a compute
        # engine (one DVE sem, ticks subsumed after the first wait), never
        # directly by DMA (8 rotating queue sems).
        if m1s:
            STst = singles.tile([128, 2, NCHUNKS, N], f16)
            for h in range(2):
                for j in range(NCHUNKS):
                    nc.sync.dma_start(
                        out=STst[:, h, j, :], in_=ST[h, j * 128:(j + 1) * 128, :]
                    )
            SThi = singles.tile([128, NCHUNKS, N], f16)
            STlo = singles.tile([128, NCHUNKS, N], f16)
            for j in range(NCHUNKS):
                nc.vector.tensor_copy(out=SThi[:, j, :], in_=STst[:, 0, j, :])
                nc.vector.tensor_copy(out=STlo[:, j, :], in_=STst[:, 1, j, :])
        else:
            STst = singles.tile([128, NCHUNKS, N], wdt)
            for j in range(NCHUNKS):
                nc.sync.dma_start(
                    out=STst[:, j, :], in_=ST[j * 128:(j + 1) * 128, :]
                )
        if m1s:
            Wdst = singles.tile([R, 2, R], f16)
            Bdst = singles.tile([R, 2, S16], f16)
            for h in range(2):
                nc.sync.dma_start(out=Wdst[:, h, :], in_=Wd[h])
                nc.sync.dma_start(out=Bdst[:, h, :], in_=Bd[h])
        else:
            Wdst = singles.tile([R, R], wdt)
            nc.sync.dma_start(out=Wdst, in_=Wd)
            Bdst = singles.tile([R, S16], wdt)
            nc.sync.dma_start(out=Bdst, in_=Bd)
        Adst = singles.tile([R, R], f32 if m1s else wdt)
        nc.sync.dma_start(out=Adst, in_=Ad)
        Xst = singles.tile([128, N], f32)
        nc.sync.dma_start(out=Xst, in_=X0T)           # host-padded to 128 rows

        if not m1s:
            STsb = singles.tile([128, NCHUNKS, N], wdt)  # ST[jc*128+p, i]
            for j in range(NCHUNKS):
                nc.vector.tensor_copy(
                    out=_cast(STsb[:, j, :], fast_dt), in_=STst[:, j, :]
                )
        if m1s:
            Wdhi = singles.tile([R, R], f16)
            Wdlo = singles.tile([R, R], f16)
            nc.vector.tensor_copy(out=Wdhi, in_=Wdst[:, 0, :])
            nc.vector.tensor_copy(out=Wdlo, in_=Wdst[:, 1, :])
            Bdhi = singles.tile([R, S16], f16)
            Bdlo = singles.tile([R, S16], f16)
            nc.vector.tensor_copy(out=Bdhi, in_=Bdst[:, 0, :])
            nc.vector.tensor_copy(out=Bdlo, in_=Bdst[:, 1, :])
            Wdsb = Bdsb = None
        else:
            Wdsb = singles.tile([R, R], wdt)
            nc.vector.tensor_copy(out=Wdsb, in_=Wdst)
            Bdsb = singles.tile([R, S16], wdt)
            nc.vector.tensor_copy(out=_cast(Bdsb, fast_dt), in_=Bdst)
        Adsb = singles.tile([R, R], f32 if m1s else wdt)  # rows >= S16 zero (host)
        nc.vector.tensor_copy(out=_cast(Adsb, fast_dt), in_=Adst)
        Xsb = singles.tile([128, N], f32)             # x^T state; rows >= S16 stay 0
        nc.vector.tensor_copy(out=Xsb, in_=Xst)
        if fp16 or fast_dt is not None:
            # reduced-precision shadow of x^T for the M3 rhs (Xsb itself must
            # stay exact fp32 — it is the output trajectory state).
            Xr = singles.tile([128, N], wdt)
            nc.vector.tensor_copy(out=_cast(Xr, fast_dt), in_=Xst)
        else:
            Xr = Xsb
        bias0 = singles.tile([128, 1], f32)
        nc.vector.memset(bias0, 0.0)

        if m1s:
            # V = Z@Wd node-major, split-fp16: Vhi = f16(V), Vlo = f16((V-Vhi)*2^11)
            Vhi = singles.tile([128, N], f16)
            Vlo = singles.tile([128, N], f16)
            Vsb = None
        else:
            Vsb = singles.tile([128, N], wdt)         # V = Z@Wd node-major:
            #   Vsb[p, c*128+r] = V[c*128+p, r]; written every step before reads.

        nc.sync.dma_start(out=xout[0], in_=Xsb[:S16, :])

        # --- recurrence -------------------------------------------------------
        for t in range(T):
            # Pb[f] = ((S @ V_t) + X_t^T-contraction)^T  in h-major, psum
            Pb = []
            for f in range(NF):
                fsl = slice(f * F, (f + 1) * F)
                pb = pP.tile([128, F], f32)
                if t > 0 and m1s:
                    for j in range(NCHUNKS):
                        nc.tensor.matmul(
                            pb,
                            lhsT=Vhi[:, j * 128:(j + 1) * 128],
                            rhs=SThi[:, j, fsl],
                            start=(j == 0),
                            stop=False,
                        )
                    for j in range(NCHUNKS):
                        nc.tensor.matmul(
                            pb,
                            lhsT=Vlo[:, j * 128:(j + 1) * 128],
                            rhs=SThi[:, j, fsl],
                            start=False,
                            stop=False,
                        )
                    nc.tensor.matmul(
                        pb, lhsT=Adsb, rhs=Xr[:, fsl], start=False, stop=True,
                    )
                    lo = pLO.tile([128, F], f32)
                    for j in range(NCHUNKS):
                        nc.tensor.matmul(
                            lo,
                            lhsT=Vhi[:, j * 128:(j + 1) * 128],
                            rhs=STlo[:, j, fsl],
                            start=(j == 0),
                            stop=(j == NCHUNKS - 1),
                        )
                    # arg = HI + LO/2^11  (SBUF, feeds tanh)
                    argsb = apool.tile([128, F], f32, tag="argsb")
                    nc.scalar.mul(out=argsb, in_=lo, mul=1.0 / LOSCALE)
                    nc.vector.tensor_add(out=argsb, in0=argsb, in1=pb)
                    Pb.append(argsb)
                    continue
                elif t > 0:
                    for j in range(NCHUNKS):
                        nc.tensor.matmul(
                            pb,
                            lhsT=_cast(Vsb[:, j * 128:(j + 1) * 128], fast_dt),
                            rhs=_cast(STsb[:, j, fsl], fast_dt),
                            start=(j == 0),
                            stop=False,
                        )
                    nc.tensor.matmul(
                        pb,
                        lhsT=_cast(Adsb, fast_dt),
                        rhs=_cast(Xr[:, fsl], fast_dt),
                        start=False,
                        stop=True,
                    )
                else:
                    nc.tensor.matmul(
                        pb,
                        lhsT=_cast(Adsb, fast_dt),
                        rhs=_cast(Xr[:, fsl], fast_dt),
                        start=True,
                        stop=True,
                    )
                Pb.append(pb)

            # Z_{t+1}^T = tanh(Pb)   (h-major, SBUF)
            Znew = zpool.tile([128, N], wdt)
            if m1s:
                Zhi = zpool.tile([128, N], f16, tag="zhi")
                Zlo = zpool.tile([128, N], f16, tag="zlo")
            for f in range(NF):
                fsl = slice(f * F, (f + 1) * F)
                nc.scalar.activation(
                    out=_cast(Znew[:, fsl], fast_dt), in_=Pb[f],
                    func=Tanh, bias=bias0, scale=1.0,
                )
                if m1s:
                    nc.vector.tensor_copy(out=Zhi[:, fsl], in_=Znew[:, fsl])
                    nc.vector.tensor_tensor(
                        out=Zlo[:, fsl], in0=Znew[:, fsl], in1=Zhi[:, fsl],
                        op=mybir.AluOpType.subtract,
                    )

            # u_t^T = Bd^T @ Znew^T   (psum, 16 partitions)
            ub = []
            for f in range(NF):
                fsl = slice(f * F, (f + 1) * F)
                u = pU.tile([S16, F], f32)
                if m1s:
                    nc.tensor.matmul(
                        u, lhsT=Bdhi, rhs=Zhi[:, fsl], start=True, stop=False)
                    nc.tensor.matmul(
                        u, lhsT=Bdlo, rhs=Zhi[:, fsl], start=False, stop=False)
                    nc.tensor.matmul(
                        u, lhsT=Bdhi, rhs=Zlo[:, fsl], start=False, stop=True)
                else:
                    nc.tensor.matmul(
                        u,
                        lhsT=_cast(Bdsb, fast_dt),
                        rhs=_cast(Znew[:, fsl], fast_dt),
                        start=True,
                        stop=True,
                    )
                ub.append(u)

            # x_{t+1}^T = x_t^T + u_t^T ; stream both trajectories out
            u_sb = upool.tile([S16, N], f32)
            for f in range(NF):
                fsl = slice(f * F, (f + 1) * F)
                nc.scalar.copy(out=u_sb[:, fsl], in_=ub[f])
                if Xr is not Xsb:
                    nc.vector.tensor_add(
                        out=_cast(Xr[:S16, fsl], fast_dt),
                        in0=Xsb[:S16, fsl], in1=ub[f],
                    )
                nc.vector.tensor_add(
                    out=Xsb[:S16, fsl], in0=Xsb[:S16, fsl], in1=ub[f]
                )
            nc.sync.dma_start(out=uout[t], in_=u_sb)
            nc.sync.dma_start(out=xout[t + 1], in_=Xsb[:S16, :])

            # V_{t+1} = Z_{t+1} @ Wd  (node-major), psum -> SBUF
            if t < T - 1:
                for g in range(NF):
                    vp = pV.tile([128, F], f32)
                    for cc in range(F // 128):
                        c = g * (F // 128) + cc
                        csl = slice(c * 128, (c + 1) * 128)
                        osl = slice(cc * 128, (cc + 1) * 128)
                        if m1s:
                            nc.tensor.matmul(
                                vp[:, osl], lhsT=Zhi[:, csl], rhs=Wdhi,
                                start=True, stop=False)
                            nc.tensor.matmul(
                                vp[:, osl], lhsT=Zlo[:, csl], rhs=Wdhi,
                                start=False, stop=False)
                            nc.tensor.matmul(
                                vp[:, osl], lhsT=Zhi[:, csl], rhs=Wdlo,
                                start=False, stop=True)
                        else:
                            nc.tensor.matmul(
                                vp[:, osl],
                                lhsT=Znew[:, csl],
                                rhs=Wdsb,
                                start=True,
                                stop=True,
                            )
                    gsl = slice(g * F, (g + 1) * F)
                    if m1s:
                        nc.vector.tensor_copy(out=Vhi[:, gsl], in_=vp)
                        nc.vector.tensor_tensor(
                            out=Vlo[:, gsl], in0=vp, in1=Vhi[:, gsl],
                            op=mybir.AluOpType.subtract,
                        )
                    else:
                        nc.vector.tensor_copy(
                            out=_cast(Vsb[:, gsl], fast_dt), in_=vp
                        )


# ---------------------------------------------------------------------------
# Host side
# ---------------------------------------------------------------------------

_NC_CACHE = {}


def _get_nc():
    key = str(FAST_MODE)
    if key not in _NC_CACHE:
        _NC_CACHE[key] = build_nc(FAST_MODE)
    return _NC_CACHE[key]


def _blockdiag2(M):
    a, b = M.shape
    out = np.zeros((2 * a, 2 * b), dtype=M.dtype)
    out[:a, :b] = M
    out[a:, b:] = M
    return out


def kernel(x0, S, A, B, W):
    x0 = np.asarray(x0, dtype=np.float32)
    S = np.asarray(S, dtype=np.float32)
    A = np.asarray(A, dtype=np.float32)
    B = np.asarray(B, dtype=np.float32)
    W = np.asarray(W, dtype=np.float32)

    wnp = np.float16 if FAST_MODE == "fp16" else np.float32

    def _hilo(M, scale=1.0):
        hi = M.astype(np.float16)
        lo = ((M - hi.astype(np.float32)) * scale).astype(np.float16)
        return np.ascontiguousarray(np.stack([hi, lo]))

    if FAST_MODE == "m1split":
        ST = _hilo(np.ascontiguousarray(S.T), LOSCALE)     # (2, N, N) f16
        Wd = _hilo(_blockdiag2(W))                         # (2, 128, 128) f16
        Bd = _hilo(_blockdiag2(B))                         # (2, 128, 16) f16
        Ad = np.zeros((R, R), dtype=np.float32)
        Ad[:S16, :] = _blockdiag2(A)
    else:
        ST = np.ascontiguousarray(S.T.astype(wnp))
        Wd = np.ascontiguousarray(_blockdiag2(W).astype(wnp))
        Bd = np.ascontiguousarray(_blockdiag2(B).astype(wnp))
        Ad = np.zeros((R, R), dtype=wnp)
        Ad[:S16, :] = _blockdiag2(A).astype(wnp)

    in_maps = []
    for c in range(NCORES):
        shard = x0[c * IPC:(c + 1) * IPC]                  # (2, N, P)
        X0T = np.zeros((128, N), dtype=np.float32)
        X0T[:S16] = shard.transpose(0, 2, 1).reshape(S16, N)
        in_maps.append({"ST": ST, "Wd": Wd, "Ad": Ad, "Bd": Bd, "X0T": X0T})

    nc = _get_nc()
    res = run_bass_kernel_spmd(nc, in_maps, core_ids=list(range(NCORES)))

    x_traj = np.empty((BATCH, T + 1, N, P), dtype=np.float32)
    u_traj = np.empty((BATCH, T, N, Q), dtype=np.float32)
    for c in range(NCORES):
        xo = res.results[c]["xout"].reshape(T + 1, IPC, P, N)
        uo = res.results[c]["uout"].reshape(T, IPC, Q, N)
        x_traj[c * IPC:(c + 1) * IPC] = xo.transpose(1, 0, 3, 2)
        u_traj[c * IPC:(c + 1) * IPC] = uo.transpose(1, 0, 3, 2)
    return (x_traj, u_traj)


# revision 26
# speedup vs baseline: 1.0138x; 1.0138x over previous
"""GRNN (graph recurrent NN) Trainium2 Bass kernel.

Reference computation (per batch item b, T=64 steps, Z0 = 0):
    Z_{t+1} = tanh(S @ Z_t @ W + x_t @ A)      (N,H)
    u_t     = Z_{t+1} @ B                      (N,Q)
    x_{t+1} = x_t + u_t                        (N,P)
Outputs: x_traj (B,T+1,N,P), u_traj (B,T,N,Q).

Sharding: data-parallel over batch across 8 cores (2 items/core). The two
items are folded into a single 128-wide axis r = item*H + h, with
block-diagonal weights Wd/Ad/Bd, so every matmul uses full 128 partitions.

Layout trick: the per-core state is V_t = Z_t @ Wd kept NODE-major.  Then
    (S @ Z_t @ Wd)^T = (S @ V_t)^T      -> matmul(lhsT=V[jchunk], rhs=ST[jchunk])
which lands h-major, where the Wd/Ad/Bd contractions want it, and
    V_{t+1} = Z_{t+1} @ Wd              -> matmul(lhsT=Znew^T[nchunk], rhs=Wd)
lands node-major again.  No transposes anywhere in the loop.
"""

import os
import sys

import numpy as np

for _p in (
    "/root/.axon_site",
    "/root/.axon_site/_ro/trn_rl_repo",
    "/root/.axon_site/_ro/pypackages",
    "/opt/trn_rl_repo",
):
    if os.path.isdir(_p) and _p not in sys.path:
        sys.path.append(_p)

import concourse.bass as bass
import concourse.tile as tile
from concourse import bacc, mybir
from concourse.bass_utils import run_bass_kernel_spmd

BATCH, N, T, P, Q, H = 16, 1024, 64, 8, 8, 64
NCORES = 8
IPC = BATCH // NCORES  # items per core = 2
R = IPC * H            # folded item*H axis = 128
S16 = IPC * P          # folded item*P axis = 16
NCHUNKS = N // 128     # 8 node chunks
F = 512                # matmul moving(free)-dim chunk
NF = N // F            # 2 free chunks over nodes

f32 = mybir.dt.float32

f16 = mybir.dt.float16

# Matmul operand mode:
#   None    — exact fp32 matmuls (4 cyc/row on PE): ~1.27 ms measured.
#   "fp16"  — fp16 operands (1 cyc/row, 11-bit mantissa).  All matmul
#             operands (S^T, V, Znew, Wd/Ad/Bd, x-shadow) stored fp16;
#             psum accumulation and the x trajectory state stay fp32.
#   "m1split" — S-matmul in 3-pass split-fp16 (hi/lo, lo prescaled 2^11,
#             ~22-bit effective operands); M3/M4/M2' stay exact fp32.
#             The recurrence amplifies per-step rounding ~64x (integrator
#             feedback), so plain fp16/f32r operands land at ~2e-2 final
#             error; the split keeps it ~1e-4 while cutting the dominant
#             matmul from 4 to 3 cyc/row total.
#   float32r (bitcast mode) exists in the code but yields wrong results on
#   HW via this Bacc path (standalone-ldweights f32r bug) — do not use.
FAST_MODE = "m1split"
LOSCALE = 2048.0


def _cast(ap, dt):
    return ap if dt is None else ap.bitcast(dt)


def build_nc(mode=FAST_MODE):
    # Bacc (not raw Bass): its compile() splits sync waits to the 1-per-
    # instruction hardware limit and moves matmul waits onto ldweights.
    nc = bacc.Bacc(trn_type="TRN2", target_bir_lowering=False, debug=False)

    wdt = f16 if mode == "fp16" else f32
    if mode == "m1split":
        ST = nc.dram_tensor("ST", [2, N, N], f16, kind="ExternalInput").ap()
        Wd = nc.dram_tensor("Wd", [2, R, R], f16, kind="ExternalInput").ap()
        Bd = nc.dram_tensor("Bd", [2, R, S16], f16, kind="ExternalInput").ap()
    else:
        ST = nc.dram_tensor("ST", [N, N], wdt, kind="ExternalInput").ap()
        Wd = nc.dram_tensor("Wd", [R, R], wdt, kind="ExternalInput").ap()
        Bd = nc.dram_tensor("Bd", [R, S16], wdt, kind="ExternalInput").ap()
    Ad = nc.dram_tensor("Ad", [R, R], f32 if mode == "m1split" else wdt,
                        kind="ExternalInput").ap()
    X0T = nc.dram_tensor("X0T", [128, N], f32, kind="ExternalInput").ap()
    xout = nc.dram_tensor("xout", [T + 1, S16, N], f32, kind="ExternalOutput").ap()
    uout = nc.dram_tensor("uout", [T, S16, N], f32, kind="ExternalOutput").ap()

    with tile.TileContext(nc) as tc:
        _grnn(tc, ST, Wd, Ad, Bd, X0T, xout, uout, mode)
    nc.compile()
    return nc


def _grnn(tc, ST, Wd, Ad, Bd, X0T, xout, uout, mode):
    nc = tc.nc
    fp16 = mode == "fp16"
    m1s = mode == "m1split"
    wdt = f16 if fp16 else f32           # matmul operand storage dtype
    fast_dt = None if (fp16 or m1s or mode is None) else mode  # f32r bitcast
    Tanh = mybir.ActivationFunctionType.Tanh

    with (
        tc.tile_pool(name="singles", bufs=1) as singles,
        tc.tile_pool(name="znew", bufs=2) as zpool,
        tc.tile_pool(name="usb", bufs=2) as upool,
        tc.tile_pool(name="arg", bufs=2) as apool,
        tc.tile_pool(name="vtmp", bufs=2) as vpool,
        tc.tile_pool(name="pP", bufs=2, space="PSUM") as pP,
        tc.tile_pool(name="pU", bufs=2, space="PSUM") as pU,
        tc.tile_pool(name="pV", bufs=2, space="PSUM") as pV,
        tc.tile_pool(name="pLO", bufs=2, space="PSUM") as pLO,
    ):
        # --- resident tensors -------------------------------------------------
        # DMA loads land in staging tiles; one-time DVE copies produce the
        # resident operand tiles.  PE matmuls must carry at most ONE sync wait
        # (LW-struct codegen limit), and Tile's wait elision is not transitive
        # across procs — so matmul operands must be produced by a compute
        # engine (one DVE sem, ticks subsumed after the first wait), never
        # directly by DMA (8 rotating queue sems).
        if m1s:
            STst = singles.tile([128, 2, NCHUNKS, N], f16)
            for h in range(2):
                for j in range(NCHUNKS):
                    nc.sync.dma_start(
                        out=STst[:, h, j, :], in_=ST[h, j * 128:(j + 1) * 128, :]
                    )
            SThi = singles.tile([128, NCHUNKS, N], f16)
            STlo = singles.tile([128, NCHUNKS, N], f16)
            for j in range(NCHUNKS):
                nc.vector.tensor_copy(out=SThi[:, j, :], in_=STst[:, 0, j, :])
                nc.vector.tensor_copy(out=STlo[:, j, :], in_=STst[:, 1, j, :])
        else:
            STst = singles.tile([128, NCHUNKS, N], wdt)
            for j in range(NCHUNKS):
                nc.sync.dma_start(
                    out=STst[:, j, :], in_=ST[j * 128:(j + 1) * 128, :]
                )
        if m1s:
            Wdst = singles.tile([R, 2, R], f16)
            Bdst = singles.tile([R, 2, S16], f16)
            for h in range(2):
                nc.sync.dma_start(out=Wdst[:, h, :], in_=Wd[h])
                nc.sync.dma_start(out=Bdst[:, h, :], in_=Bd[h])
        else:
            Wdst = singles.tile([R, R], wdt)
            nc.sync.dma_start(out=Wdst, in_=Wd)
            Bdst = singles.tile([R, S16], wdt)
            nc.sync.dma_start(out=Bdst, in_=Bd)
        Adst = singles.tile([R, R], f32 if m1s else wdt)
        nc.sync.dma_start(out=Adst, in_=Ad)
        Xst = singles.tile([128, N], f32)
        nc.sync.dma_start(out=Xst, in_=X0T)           # host-padded to 128 rows

        if not m1s:
            STsb = singles.tile([128, NCHUNKS, N], wdt)  # ST[jc*128+p, i]
            for j in range(NCHUNKS):
                nc.vector.tensor_copy(
                    out=_cast(STsb[:, j, :], fast_dt), in_=STst[:, j, :]
                )
        if m1s:
            # exact fp32 Wd/Bd reconstructed from the hi+lo fp16 pair
            Wdsb = singles.tile([R, R], f32)
            nc.vector.tensor_tensor(
                out=Wdsb, in0=Wdst[:, 0, :], in1=Wdst[:, 1, :],
                op=mybir.AluOpType.add)
            Bdsb = singles.tile([R, S16], f32)
            nc.vector.tensor_tensor(
                out=Bdsb, in0=Bdst[:, 0, :], in1=Bdst[:, 1, :],
                op=mybir.AluOpType.add)
        else:
            Wdsb = singles.tile([R, R], wdt)
            nc.vector.tensor_copy(out=Wdsb, in_=Wdst)
            Bdsb = singles.tile([R, S16], wdt)
            nc.vector.tensor_copy(out=_cast(Bdsb, fast_dt), in_=Bdst)
        Adsb = singles.tile([R, R], f32 if m1s else wdt)  # rows >= S16 zero (host)
        nc.vector.tensor_copy(out=_cast(Adsb, fast_dt), in_=Adst)
        Xsb = singles.tile([128, N], f32)             # x^T state; rows >= S16 stay 0
        nc.vector.tensor_copy(out=Xsb, in_=Xst)
        if fp16 or fast_dt is not None:
            # reduced-precision shadow of x^T for the M3 rhs (Xsb itself must
            # stay exact fp32 — it is the output trajectory state).
            Xr = singles.tile([128, N], wdt)
            nc.vector.tensor_copy(out=_cast(Xr, fast_dt), in_=Xst)
        else:
            Xr = Xsb
        bias0 = singles.tile([128, 1], f32)
        nc.vector.memset(bias0, 0.0)

        if m1s:
            # V = Z@Wd node-major, split-fp16: Vhi = f16(V), Vlo = f16((V-Vhi)*2^11)
            Vhi = singles.tile([128, N], f16)
            Vlo = singles.tile([128, N], f16)
            Vsb = None
        else:
            Vsb = singles.tile([128, N], wdt)         # V = Z@Wd node-major:
            #   Vsb[p, c*128+r] = V[c*128+p, r]; written every step before reads.

        nc.sync.dma_start(out=xout[0], in_=Xsb[:S16, :])

        # --- recurrence -------------------------------------------------------
        for t in range(T):
            # Pb[f] = ((S @ V_t) + X_t^T-contraction)^T  in h-major, psum
            Pb = []
            for f in range(NF):
                fsl = slice(f * F, (f + 1) * F)
                pb = pP.tile([128, F], f32)
                if t > 0 and m1s:
                    for j in range(NCHUNKS):
                        nc.tensor.matmul(
                            pb,
                            lhsT=Vhi[:, j * 128:(j + 1) * 128],
                            rhs=SThi[:, j, fsl],
                            start=(j == 0),
                            stop=False,
                        )
                    nc.tensor.matmul(
                        pb, lhsT=Adsb, rhs=Xr[:, fsl], start=False, stop=True,
                    )
                    lo = pLO.tile([128, F], f32)
                    for j in range(NCHUNKS):
                        nc.tensor.matmul(
                            lo,
                            lhsT=Vhi[:, j * 128:(j + 1) * 128],
                            rhs=STlo[:, j, fsl],
                            start=(j == 0),
                            stop=False,
                        )
                    for j in range(NCHUNKS):
                        nc.tensor.matmul(
                            lo,
                            lhsT=Vlo[:, j * 128:(j + 1) * 128],
                            rhs=SThi[:, j, fsl],
                            start=False,
                            stop=(j == NCHUNKS - 1),
                        )
                    # arg = HI + LO/2^11  (SBUF, feeds tanh)
                    argsb = apool.tile([128, F], f32, tag="argsb")
                    nc.scalar.mul(out=argsb, in_=lo, mul=1.0 / LOSCALE)
                    nc.vector.tensor_add(out=argsb, in0=argsb, in1=pb)
                    Pb.append(argsb)
                    continue
                elif t > 0:
                    for j in range(NCHUNKS):
                        nc.tensor.matmul(
                            pb,
                            lhsT=_cast(Vsb[:, j * 128:(j + 1) * 128], fast_dt),
                            rhs=_cast(STsb[:, j, fsl], fast_dt),
                            start=(j == 0),
                            stop=False,
                        )
                    nc.tensor.matmul(
                        pb,
                        lhsT=_cast(Adsb, fast_dt),
                        rhs=_cast(Xr[:, fsl], fast_dt),
                        start=False,
                        stop=True,
                    )
                else:
                    nc.tensor.matmul(
                        pb,
                        lhsT=_cast(Adsb, fast_dt),
                        rhs=_cast(Xr[:, fsl], fast_dt),
                        start=True,
                        stop=True,
                    )
                Pb.append(pb)

            # Z_{t+1}^T = tanh(Pb)   (h-major, SBUF)
            Znew = zpool.tile([128, N], wdt)
            for f in range(NF):
                fsl = slice(f * F, (f + 1) * F)
                nc.scalar.activation(
                    out=_cast(Znew[:, fsl], fast_dt), in_=Pb[f],
                    func=Tanh, bias=bias0, scale=1.0,
                )

            # u_t^T = Bd^T @ Znew^T   (psum, 16 partitions)
            ub = []
            for f in range(NF):
                fsl = slice(f * F, (f + 1) * F)
                u = pU.tile([S16, F], f32)
                nc.tensor.matmul(
                    u,
                    lhsT=_cast(Bdsb, fast_dt),
                    rhs=_cast(Znew[:, fsl], fast_dt),
                    start=True,
                    stop=True,
                )
                ub.append(u)

            # x_{t+1}^T = x_t^T + u_t^T ; stream both trajectories out
            u_sb = upool.tile([S16, N], f32)
            for f in range(NF):
                fsl = slice(f * F, (f + 1) * F)
                nc.scalar.copy(out=u_sb[:, fsl], in_=ub[f])
                if Xr is not Xsb:
                    nc.vector.tensor_add(
                        out=_cast(Xr[:S16, fsl], fast_dt),
                        in0=Xsb[:S16, fsl], in1=ub[f],
                    )
                nc.vector.tensor_add(
                    out=Xsb[:S16, fsl], in0=Xsb[:S16, fsl], in1=ub[f]
                )
            nc.sync.dma_start(out=uout[t], in_=u_sb)
            nc.sync.dma_start(out=xout[t + 1], in_=Xsb[:S16, :])

            # V_{t+1} = Z_{t+1} @ Wd  (node-major), psum -> SBUF
            if t < T - 1:
                for g in range(NF):
                    vp = pV.tile([128, F], f32)
                    for cc in range(F // 128):
                        c = g * (F // 128) + cc
                        csl = slice(c * 128, (c + 1) * 128)
                        osl = slice(cc * 128, (cc + 1) * 128)
                        nc.tensor.matmul(
                            vp[:, osl],
                            lhsT=Znew[:, csl],
                            rhs=Wdsb,
                            start=True,
                            stop=True,
                        )
                    gsl = slice(g * F, (g + 1) * F)
                    if m1s:
                        nc.vector.tensor_copy(out=Vhi[:, gsl], in_=vp)
                        vt = vpool.tile([128, F], f32, tag="vt")
                        nc.vector.tensor_tensor(
                            out=vt, in0=vp, in1=Vhi[:, gsl],
                            op=mybir.AluOpType.subtract,
                        )
                        nc.vector.tensor_scalar_mul(
                            out=Vlo[:, gsl], in0=vt, scalar1=LOSCALE
                        )
                    else:
                        nc.vector.tensor_copy(
                            out=_cast(Vsb[:, gsl], fast_dt), in_=vp
                        )


# ---------------------------------------------------------------------------
# Host side
# ---------------------------------------------------------------------------

_NC_CACHE = {}


def _get_nc():
    key = str(FAST_MODE)
    if key not in _NC_CACHE:
        _NC_CACHE[key] = build_nc(FAST_MODE)
    return _NC_CACHE[key]


def _blockdiag2(M):
    a, b = M.shape
    out = np.zeros((2 * a, 2 * b), dtype=M.dtype)
    out[:a, :b] = M
    out[a:, b:] = M
    return out


def kernel(x0, S, A, B, W):
    x0 = np.asarray(x0, dtype=np.float32)
    S = np.asarray(S, dtype=np.float32)
    A = np.asarray(A, dtype=np.float32)
    B = np.asarray(B, dtype=np.float32)
    W = np.asarray(W, dtype=np.float32)

    wnp = np.float16 if FAST_MODE == "fp16" else np.float32

    def _hilo(M, scale=1.0):
        hi = M.astype(np.float16)
        lo = ((M - hi.astype(np.float32)) * scale).astype(np.float16)
        return np.ascontiguousarray(np.stack([hi, lo]))

    if FAST_MODE == "m1split":
        ST = _hilo(np.ascontiguousarray(S.T), LOSCALE)     # (2, N, N) f16
        Wd = _hilo(_blockdiag2(W))                         # (2, 128, 128) f16
        Bd = _hilo(_blockdiag2(B))                         # (2, 128, 16) f16
        Ad = np.zeros((R, R), dtype=np.float32)
        Ad[:S16, :] = _blockdiag2(A)
    else:
        ST = np.ascontiguousarray(S.T.astype(wnp))
        Wd = np.ascontiguousarray(_blockdiag2(W).astype(wnp))
        Bd = np.ascontiguousarray(_blockdiag2(B).astype(wnp))
        Ad = np.zeros((R, R), dtype=wnp)
        Ad[:S16, :] = _blockdiag2(A).astype(wnp)

    in_maps = []
    for c in range(NCORES):
        shard = x0[c * IPC:(c + 1) * IPC]                  # (2, N, P)
        X0T = np.zeros((128, N), dtype=np.float32)
        X0T[:S16] = shard.transpose(0, 2, 1).reshape(S16, N)
        in_maps.append({"ST": ST, "Wd": Wd, "Ad": Ad, "Bd": Bd, "X0T": X0T})

    nc = _get_nc()
    res = run_bass_kernel_spmd(nc, in_maps, core_ids=list(range(NCORES)))

    x_traj = np.empty((BATCH, T + 1, N, P), dtype=np.float32)
    u_traj = np.empty((BATCH, T, N, Q), dtype=np.float32)
    for c in range(NCORES):
        xo = res.results[c]["xout"].reshape(T + 1, IPC, P, N)
        uo = res.results[c]["uout"].reshape(T, IPC, Q, N)
        x_traj[c * IPC:(c + 1) * IPC] = xo.transpose(1, 0, 3, 2)
        u_traj[c * IPC:(c + 1) * IPC] = uo.transpose(1, 0, 3, 2)
    return (x_traj, u_traj)


# revision 27
# speedup vs baseline: 1.0156x; 1.0018x over previous
"""GRNN (graph recurrent NN) Trainium2 Bass kernel.

Reference computation (per batch item b, T=64 steps, Z0 = 0):
    Z_{t+1} = tanh(S @ Z_t @ W + x_t @ A)      (N,H)
    u_t     = Z_{t+1} @ B                      (N,Q)
    x_{t+1} = x_t + u_t                        (N,P)
Outputs: x_traj (B,T+1,N,P), u_traj (B,T,N,Q).

Sharding: data-parallel over batch across 8 cores (2 items/core). The two
items are folded into a single 128-wide axis r = item*H + h, with
block-diagonal weights Wd/Ad/Bd, so every matmul uses full 128 partitions.

Layout trick: the per-core state is V_t = Z_t @ Wd kept NODE-major.  Then
    (S @ Z_t @ Wd)^T = (S @ V_t)^T      -> matmul(lhsT=V[jchunk], rhs=ST[jchunk])
which lands h-major, where the Wd/Ad/Bd contractions want it, and
    V_{t+1} = Z_{t+1} @ Wd              -> matmul(lhsT=Znew^T[nchunk], rhs=Wd)
lands node-major again.  No transposes anywhere in the loop.
"""

import os
import sys

import numpy as np

for _p in (
    "/root/.axon_site",
    "/root/.axon_site/_ro/trn_rl_repo",
    "/root/.axon_site/_ro/pypackages",
    "/opt/trn_rl_repo",
):
    if os.path.isdir(_p) and _p not in sys.path:
        sys.path.append(_p)

import concourse.bass as bass
import concourse.tile as tile
from concourse import bacc, mybir
from concourse.bass_utils import run_bass_kernel_spmd

BATCH, N, T, P, Q, H = 16, 1024, 64, 8, 8, 64
NCORES = 8
IPC = BATCH // NCORES  # items per core = 2
R = IPC * H            # folded item*H axis = 128
S16 = IPC * P          # folded item*P axis = 16
NCHUNKS = N // 128     # 8 node chunks
F = 512                # matmul moving(free)-dim chunk
NF = N // F            # 2 free chunks over nodes

f32 = mybir.dt.float32

f16 = mybir.dt.float16

# Matmul operand mode:
#   None    — exact fp32 matmuls (4 cyc/row on PE): ~1.27 ms measured.
#   "fp16"  — fp16 operands (1 cyc/row, 11-bit mantissa).  All matmul
#             operands (S^T, V, Znew, Wd/Ad/Bd, x-shadow) stored fp16;
#             psum accumulation and the x trajectory state stay fp32.
#   "m1split" — S-matmul in 3-pass split-fp16 (hi/lo, lo prescaled 2^11,
#             ~22-bit effective operands); M3/M4/M2' stay exact fp32.
#             The recurrence amplifies per-step rounding ~64x (integrator
#             feedback), so plain fp16/f32r operands land at ~2e-2 final
#             error; the split keeps it ~1e-4 while cutting the dominant
#             matmul from 4 to 3 cyc/row total.
#   float32r (bitcast mode) exists in the code but yields wrong results on
#   HW via this Bacc path (standalone-ldweights f32r bug) — do not use.
FAST_MODE = "m1split"
LOSCALE = 2048.0


def _cast(ap, dt):
    return ap if dt is None else ap.bitcast(dt)


def build_nc(mode=FAST_MODE):
    # Bacc (not raw Bass): its compile() splits sync waits to the 1-per-
    # instruction hardware limit and moves matmul waits onto ldweights.
    nc = bacc.Bacc(trn_type="TRN2", target_bir_lowering=False, debug=False)

    wdt = f16 if mode == "fp16" else f32
    if mode == "m1split":
        ST = nc.dram_tensor("ST", [2, N, N], f16, kind="ExternalInput").ap()
    else:
        ST = nc.dram_tensor("ST", [N, N], wdt, kind="ExternalInput").ap()
    Wd = nc.dram_tensor("Wd", [R, R], wdt, kind="ExternalInput").ap()
    Bd = nc.dram_tensor("Bd", [R, S16], wdt, kind="ExternalInput").ap()
    Ad = nc.dram_tensor("Ad", [R, R], wdt, kind="ExternalInput").ap()
    X0T = nc.dram_tensor("X0T", [128, N], f32, kind="ExternalInput").ap()
    xout = nc.dram_tensor("xout", [T + 1, S16, N], f32, kind="ExternalOutput").ap()
    uout = nc.dram_tensor("uout", [T, S16, N], f32, kind="ExternalOutput").ap()

    with tile.TileContext(nc) as tc:
        _grnn(tc, ST, Wd, Ad, Bd, X0T, xout, uout, mode)
    nc.compile()
    return nc


def _grnn(tc, ST, Wd, Ad, Bd, X0T, xout, uout, mode):
    nc = tc.nc
    fp16 = mode == "fp16"
    m1s = mode == "m1split"
    wdt = f16 if fp16 else f32           # matmul operand storage dtype
    fast_dt = None if (fp16 or m1s or mode is None) else mode  # f32r bitcast
    Tanh = mybir.ActivationFunctionType.Tanh

    with (
        tc.tile_pool(name="singles", bufs=1) as singles,
        tc.tile_pool(name="znew", bufs=2) as zpool,
        tc.tile_pool(name="usb", bufs=2) as upool,
        tc.tile_pool(name="arg", bufs=2) as apool,
        tc.tile_pool(name="vtmp", bufs=2) as vpool,
        tc.tile_pool(name="pP", bufs=2, space="PSUM") as pP,
        tc.tile_pool(name="pU", bufs=2, space="PSUM") as pU,
        tc.tile_pool(name="pV", bufs=2, space="PSUM") as pV,
        tc.tile_pool(name="pLO", bufs=2, space="PSUM") as pLO,
    ):
        # --- resident tensors -------------------------------------------------
        # DMA loads land in staging tiles; one-time DVE copies produce the
        # resident operand tiles.  PE matmuls must carry at most ONE sync wait
        # (LW-struct codegen limit), and Tile's wait elision is not transitive
        # across procs — so matmul operands must be produced by a compute
        # engine (one DVE sem, ticks subsumed after the first wait), never
        # directly by DMA (8 rotating queue sems).
        if m1s:
            STst = singles.tile([128, 2, NCHUNKS, N], f16)
            for h in range(2):
                for j in range(NCHUNKS):
                    nc.sync.dma_start(
                        out=STst[:, h, j, :], in_=ST[h, j * 128:(j + 1) * 128, :]
                    )
            SThi = singles.tile([128, NCHUNKS, N], f16)
            STlo = singles.tile([128, NCHUNKS, N], f16)
            for j in range(NCHUNKS):
                nc.vector.tensor_copy(out=SThi[:, j, :], in_=STst[:, 0, j, :])
                nc.vector.tensor_copy(out=STlo[:, j, :], in_=STst[:, 1, j, :])
        else:
            STst = singles.tile([128, NCHUNKS, N], wdt)
            for j in range(NCHUNKS):
                nc.sync.dma_start(
                    out=STst[:, j, :], in_=ST[j * 128:(j + 1) * 128, :]
                )
        Wdst = singles.tile([R, R], wdt)
        nc.sync.dma_start(out=Wdst, in_=Wd)
        Bdst = singles.tile([R, S16], wdt)
        nc.sync.dma_start(out=Bdst, in_=Bd)
        Adst = singles.tile([R, R], wdt)
        nc.sync.dma_start(out=Adst, in_=Ad)
        Xst = singles.tile([128, N], f32)
        nc.sync.dma_start(out=Xst, in_=X0T)           # host-padded to 128 rows

        if not m1s:
            STsb = singles.tile([128, NCHUNKS, N], wdt)  # ST[jc*128+p, i]
            for j in range(NCHUNKS):
                nc.vector.tensor_copy(
                    out=_cast(STsb[:, j, :], fast_dt), in_=STst[:, j, :]
                )
        Wdsb = singles.tile([R, R], wdt)
        nc.vector.tensor_copy(out=Wdsb, in_=Wdst)
        Bdsb = singles.tile([R, S16], wdt)
        nc.vector.tensor_copy(out=_cast(Bdsb, fast_dt), in_=Bdst)
        Adsb = singles.tile([R, R], wdt)              # rows >= S16 are zero (host)
        nc.vector.tensor_copy(out=_cast(Adsb, fast_dt), in_=Adst)
        Xsb = singles.tile([128, N], f32)             # x^T state; rows >= S16 stay 0
        nc.vector.tensor_copy(out=Xsb, in_=Xst)
        if fp16 or fast_dt is not None:
            # reduced-precision shadow of x^T for the M3 rhs (Xsb itself must
            # stay exact fp32 — it is the output trajectory state).
            Xr = singles.tile([128, N], wdt)
            nc.vector.tensor_copy(out=_cast(Xr, fast_dt), in_=Xst)
        else:
            Xr = Xsb
        bias0 = singles.tile([128, 1], f32)
        nc.vector.memset(bias0, 0.0)

        if m1s:
            # V = Z@Wd node-major, split-fp16: Vhi = f16(V), Vlo = f16((V-Vhi)*2^11)
            Vhi = singles.tile([128, N], f16)
            Vlo = singles.tile([128, N], f16)
            Vsb = None
        else:
            Vsb = singles.tile([128, N], wdt)         # V = Z@Wd node-major:
            #   Vsb[p, c*128+r] = V[c*128+p, r]; written every step before reads.

        nc.sync.dma_start(out=xout[0], in_=Xsb[:S16, :])

        # --- recurrence -------------------------------------------------------
        for t in range(T):
            # Pb[f] = ((S @ V_t) + X_t^T-contraction)^T  in h-major, psum
            Pb = []
            for f in range(NF):
                fsl = slice(f * F, (f + 1) * F)
                pb = pP.tile([128, F], f32)
                if t > 0 and m1s:
                    for j in range(NCHUNKS):
                        nc.tensor.matmul(
                            pb,
                            lhsT=Vhi[:, j * 128:(j + 1) * 128],
                            rhs=SThi[:, j, fsl],
                            start=(j == 0),
                            stop=False,
                        )
                    nc.tensor.matmul(
                        pb, lhsT=Adsb, rhs=Xr[:, fsl], start=False, stop=True,
                    )
                    lo = pLO.tile([128, F], f32)
                    for j in range(NCHUNKS):
                        nc.tensor.matmul(
                            lo,
                            lhsT=Vhi[:, j * 128:(j + 1) * 128],
                            rhs=STlo[:, j, fsl],
                            start=(j == 0),
                            stop=False,
                        )
                    for j in range(NCHUNKS):
                        nc.tensor.matmul(
                            lo,
                            lhsT=Vlo[:, j * 128:(j + 1) * 128],
                            rhs=SThi[:, j, fsl],
                            start=False,
                            stop=(j == NCHUNKS - 1),
                        )
                    # arg = HI + LO/2^11  (SBUF, feeds tanh)
                    argsb = apool.tile([128, F], f32, tag="argsb")
                    nc.scalar.mul(out=argsb, in_=lo, mul=1.0 / LOSCALE)
                    nc.vector.tensor_add(out=argsb, in0=argsb, in1=pb)
                    Pb.append(argsb)
                    continue
                elif t > 0:
                    for j in range(NCHUNKS):
                        nc.tensor.matmul(
                            pb,
                            lhsT=_cast(Vsb[:, j * 128:(j + 1) * 128], fast_dt),
                            rhs=_cast(STsb[:, j, fsl], fast_dt),
                            start=(j == 0),
                            stop=False,
                        )
                    nc.tensor.matmul(
                        pb,
                        lhsT=_cast(Adsb, fast_dt),
                        rhs=_cast(Xr[:, fsl], fast_dt),
                        start=False,
                        stop=True,
                    )
                else:
                    nc.tensor.matmul(
                        pb,
                        lhsT=_cast(Adsb, fast_dt),
                        rhs=_cast(Xr[:, fsl], fast_dt),
                        start=True,
                        stop=True,
                    )
                Pb.append(pb)

            # Z_{t+1}^T = tanh(Pb)   (h-major, SBUF)
            Znew = zpool.tile([128, N], wdt)
            for f in range(NF):
                fsl = slice(f * F, (f + 1) * F)
                nc.scalar.activation(
                    out=_cast(Znew[:, fsl], fast_dt), in_=Pb[f],
                    func=Tanh, bias=bias0, scale=1.0,
                )

            # u_t^T = Bd^T @ Znew^T   (psum, 16 partitions)
            ub = []
            for f in range(NF):
                fsl = slice(f * F, (f + 1) * F)
                u = pU.tile([S16, F], f32)
                nc.tensor.matmul(
                    u,
                    lhsT=_cast(Bdsb, fast_dt),
                    rhs=_cast(Znew[:, fsl], fast_dt),
                    start=True,
                    stop=True,
                )
                ub.append(u)

            # x_{t+1}^T = x_t^T + u_t^T ; stream both trajectories out
            u_sb = upool.tile([S16, N], f32)
            for f in range(NF):
                fsl = slice(f * F, (f + 1) * F)
                nc.scalar.copy(out=u_sb[:, fsl], in_=ub[f])
                if Xr is not Xsb:
                    nc.vector.tensor_add(
                        out=_cast(Xr[:S16, fsl], fast_dt),
                        in0=Xsb[:S16, fsl], in1=ub[f],
                    )
                nc.vector.tensor_add(
                    out=Xsb[:S16, fsl], in0=Xsb[:S16, fsl], in1=ub[f]
                )
            nc.sync.dma_start(out=uout[t], in_=u_sb)
            nc.sync.dma_start(out=xout[t + 1], in_=Xsb[:S16, :])

            # V_{t+1} = Z_{t+1} @ Wd  (node-major), psum -> SBUF
            if t < T - 1:
                for g in range(NF):
                    vp = pV.tile([128, F], f32)
                    for cc in range(F // 128):
                        c = g * (F // 128) + cc
                        csl = slice(c * 128, (c + 1) * 128)
                        osl = slice(cc * 128, (cc + 1) * 128)
                        nc.tensor.matmul(
                            vp[:, osl],
                            lhsT=Znew[:, csl],
                            rhs=Wdsb,
                            start=True,
                            stop=True,
                        )
                    gsl = slice(g * F, (g + 1) * F)
                    if m1s:
                        nc.vector.tensor_copy(out=Vhi[:, gsl], in_=vp)
                        vt = vpool.tile([128, F], f32, tag="vt")
                        nc.vector.tensor_tensor(
                            out=vt, in0=vp, in1=Vhi[:, gsl],
                            op=mybir.AluOpType.subtract,
                        )
                        nc.vector.tensor_scalar_mul(
                            out=Vlo[:, gsl], in0=vt, scalar1=LOSCALE
                        )
                    else:
                        nc.vector.tensor_copy(
                            out=_cast(Vsb[:, gsl], fast_dt), in_=vp
                        )


# ---------------------------------------------------------------------------
# Host side
# ---------------------------------------------------------------------------

_NC_CACHE = {}


def _get_nc():
    key = str(FAST_MODE)
    if key not in _NC_CACHE:
        _NC_CACHE[key] = build_nc(FAST_MODE)
    return _NC_CACHE[key]


def _blockdiag2(M):
    a, b = M.shape
    out = np.zeros((2 * a, 2 * b), dtype=M.dtype)
    out[:a, :b] = M
    out[a:, b:] = M
    return out


def kernel(x0, S, A, B, W):
    x0 = np.asarray(x0, dtype=np.float32)
    S = np.asarray(S, dtype=np.float32)
    A = np.asarray(A, dtype=np.float32)
    B = np.asarray(B, dtype=np.float32)
    W = np.asarray(W, dtype=np.float32)

    wnp = np.float16 if FAST_MODE == "fp16" else np.float32

    def _hilo(M, scale=1.0):
        hi = M.astype(np.float16)
        lo = ((M - hi.astype(np.float32)) * scale).astype(np.float16)
        return np.ascontiguousarray(np.stack([hi, lo]))

    if FAST_MODE == "m1split":
        ST = _hilo(np.ascontiguousarray(S.T), LOSCALE)     # (2, N, N) f16
    else:
        ST = np.ascontiguousarray(S.T.astype(wnp))
    Wd = np.ascontiguousarray(_blockdiag2(W).astype(wnp))
    Bd = np.ascontiguousarray(_blockdiag2(B).astype(wnp))
    Ad = np.zeros((R, R), dtype=wnp)
    Ad[:S16, :] = _blockdiag2(A).astype(wnp)

    in_maps = []
    for c in range(NCORES):
        shard = x0[c * IPC:(c + 1) * IPC]                  # (2, N, P)
        X0T = np.zeros((128, N), dtype=np.float32)
        X0T[:S16] = shard.transpose(0, 2, 1).reshape(S16, N)
        in_maps.append({"ST": ST, "Wd": Wd, "Ad": Ad, "Bd": Bd, "X0T": X0T})

    nc = _get_nc()
    res = run_bass_kernel_spmd(nc, in_maps, core_ids=list(range(NCORES)))

    x_traj = np.empty((BATCH, T + 1, N, P), dtype=np.float32)
    u_traj = np.empty((BATCH, T, N, Q), dtype=np.float32)
    for c in range(NCORES):
        xo = res.results[c]["xout"].reshape(T + 1, IPC, P, N)
        uo = res.results[c]["uout"].reshape(T, IPC, Q, N)
        x_traj[c * IPC:(c + 1) * IPC] = xo.transpose(1, 0, 3, 2)
        u_traj[c * IPC:(c + 1) * IPC] = uo.transpose(1, 0, 3, 2)
    return (x_traj, u_traj)
